# revision 38
# baseline (speedup 1.0000x reference)
"""SAN aggregation kernel for Trainium2 (Bass/Tile), 8-core data-parallel.

Problem: out[n,c,h,w] = sum_k w[n, c//8, k, h*W+w] * xpad[n, c, h+dh(k), w+dw(k)]
  x: [8, 64, 128, 128] f32, w: [8, 8, 9, 16384] f32, 3x3 window, pad 1.

Sharding: batch dim N=8 across 8 NeuronCores (1 image per core, no
cross-core communication).

Per-core layout (everything resident in SBUF):
  partitions p = hb*8 + cw   (hb: 16 row-blocks of 8 rows, cw: 8 weight chans)
  x_sb  [128, 8*10*128]: per gl, rows [hb*8-1, hb*8+9) of channel c=cw*8+gl,
        stored 128-pitch CONTIGUOUS (each (gl, partition) block loads as one
        5 KB contiguous DMA run straight from HBM).  No column padding: the
        dw=0 / dw=2 taps simply skip the output border column whose
        x-operand would be the zero pad (their contribution is zero).
        Vertical halo rows at hb=0 / hb=15 are memset to zero.
  w_sb  [128, 9*1024]:  w[cw, k, hb-rows] per partition, k-major.

Compute: all on the Vector engine (measured: GPSIMD streaming concurrently
with DVE slows DVE ~2.6x via the shared SBUF port, so offloading loses).
Per gl: 9 tensor_mul (one per tap) + 8 tensor_add accumulate, widths
127/128 by tap so no wrap-around columns are ever read.

DMA: w planes + x mains (dep-free, big) on the sync queue in consumption
order; x edge pieces and output stores on the scalar queue.  Queue FIFOs
head-of-line block on sem-waits, so dep-free loads are kept together.
"""

import sys
import os

for _p in ("/opt/trn_rl_repo", "/root/.axon_site/_ro/trn_rl_repo"):
    if _p not in sys.path and os.path.isdir(_p):
        sys.path.append(_p)

import numpy as np

import concourse.bass as bass
import concourse.bacc as bacc
import concourse.mybir as mybir
import bass_rust
from concourse.tile import TileContext
from concourse.tile_rust import add_dep_helper

F32 = mybir.dt.float32

C, H, W = 64, 128, 128
S = H * W          # 16384
CW, GL = 8, 8      # weight channels, share planes
HB = 16            # row blocks
RB = H // HB       # rows per block = 8
XROWS = RB + 2     # 10 rows incl halo
XGL = XROWS * W    # 1280 elements per gl block in x_sb
SB = RB * W        # 1024 spatial elems per partition per gl


def _ap(base, dims, extra_offset=0):
    """Copy AP `base`, replace its [step,count] dims, bump offset.

    dims[0] is the partition dim: step "P" substitutes the base AP's own
    partition stride (flat element space, = free width).
    """
    c = base.copy()
    pstep = base.ap[0][0]
    dims = [[pstep if s == "P" else s, n] for s, n in dims]
    c.ap = bass_rust.VecI64Pair(dims)
    if extra_offset:
        c.offset = c.offset + extra_offset
    return c


def build_program():
    nc = bacc.Bacc("TRN2", target_bir_lowering=False, debug=False)
    x_d = nc.dram_tensor("x", [C, S], F32, kind="ExternalInput")
    w_d = nc.dram_tensor("w", [CW, 9, S], F32, kind="ExternalInput")
    o_d = nc.dram_tensor("out", [C, S], F32, kind="ExternalOutput")

    with TileContext(nc) as tc:
        with tc.tile_pool(name="main", bufs=1) as pool, \
             tc.tile_pool(name="qtree", bufs=2) as qpool, \
             tc.tile_pool(name="os", bufs=8) as opool:
            x_sb = pool.tile([128, GL * XGL + 4], F32)  # +guards for (dw-1)/(dw+1) taps
            w_sb = pool.tile([128, 9 * SB], F32)

            # zero the vertical halo rows that have no source data:
            # r=0 at hb=0 (partitions 0..8), r=9 at hb=15 (partitions
            # 120..128); the in-range partitions are overwritten by DMA.
            nc.vector.memset(
                _ap(x_sb[:], [["P", 128], [1, 2]]), 0.0)
            nc.vector.memset(
                _ap(x_sb[:], [["P", 128], [1, 2]],
                    extra_offset=2 + GL * XGL), 0.0)
            nc.vector.memset(
                _ap(x_sb[:], [["P", 128], [XGL, GL], [1, W]],
                    extra_offset=2), 0.0)
            nc.vector.memset(
                _ap(x_sb[:], [["P", 128], [XGL, GL], [1, W]],
                    extra_offset=2 + (XROWS - 1) * W), 0.0)

            def load_w_k(k):
                nc.sync.dma_start(
                    out=_ap(w_sb[:], [["P", 128], [1, SB]],
                            extra_offset=k * SB),
                    in_=_ap(w_d.ap(), [[SB, HB], [9 * S, CW], [1, SB]],
                            extra_offset=k * S))

            def load_x_main(gl, eng):
                # partitions 8..120 (hb 1..14): rows hb*8-1 .. hb*8+9 = one
                # 1280-element contiguous run of channel c per partition.
                eng.dma_start(
                    out=_ap(x_sb[8:120], [["P", 112], [1, XGL]],
                            extra_offset=2 + gl * XGL),
                    in_=_ap(x_d.ap(), [[RB * W, HB - 2], [GL * S, CW],
                                       [1, XGL]],
                            extra_offset=gl * S + (RB - 1) * W))

            def load_x_edges(gl, eng):
                # hb=0 (partitions 0..8): rows r=1..9 = x rows 0..8
                eng.dma_start(
                    out=_ap(x_sb[0:8], [["P", 8], [1, (XROWS - 1) * W]],
                            extra_offset=2 + gl * XGL + W),
                    in_=_ap(x_d.ap(), [[GL * S, CW], [1, (XROWS - 1) * W]],
                            extra_offset=gl * S))
                # hb=15 (partitions 120..128): rows r=0..8 = x rows 119..127
                eng.dma_start(
                    out=_ap(x_sb[120:128], [["P", 8], [1, (XROWS - 1) * W]],
                            extra_offset=2 + gl * XGL),
                    in_=_ap(x_d.ap(), [[GL * S, CW], [1, (XROWS - 1) * W]],
                            extra_offset=gl * S + (H - XROWS + 1) * W))

            # Issue order: sync = w planes + x0/x1 only (so gl0/gl1's
            # inputs and all w land without queuing behind later loads);
            # scalar = edge pieces, then x2..x7 mains, then (later) the
            # output stores.  Splitting the mains keeps each queue's
            # 9-semaphore recycle window shallow.
            # Queue/ring discipline (all measured):
            #  - scalar/ACT issues no DMAs until ~10us (framework preamble)
            #    -> gl0's working set must ride sync;
            #  - narrow 8-partition edge DMAs use only 1-2 of the 16 rings
            #    and their packets queue FIFO behind any bulk already
            #    enqueued -> issue edges BEFORE the bulk on each queue.
            load_x_edges(0, nc.sync)
            load_w_k(0)
            load_x_main(0, nc.sync)
            for k in range(1, 9):
                load_w_k(k)
            for gl in range(1, GL):
                load_x_edges(gl, nc.scalar)
            for gl in range(1, GL):
                load_x_main(gl, nc.scalar)

            # ---- compute (all DVE) ----
            # tap (dh, dw): out[h', w] += w_k[h', w] * x[r=h'+dh, w+dw-1];
            # dw=0 skips output col 0, dw=2 skips output col W-1 (their
            # x operand is the zero pad).
            def out_dma(gl, src):
                nc.scalar.dma_start(
                    out=_ap(o_d.ap(), [[RB * W, HB], [GL * S, CW], [1, SB]],
                            extra_offset=gl * S),
                    in_=src)

            # zero w border cols once (after the w loads): col 0 of the
            # dw=0 planes (k=0,3,6), col 127 of the dw=2 planes (k=2,5,8);
            # then every tap is full-width (off-image x operand hits a
            # zero weight) and the 9 product planes can be tree-reduced
            # with wide contiguous adds.
            nc.vector.memset(
                _ap(w_sb[:], [["P", 128], [3 * SB, 3], [W, RB]]), 0.0)
            nc.vector.memset(
                _ap(w_sb[:], [["P", 128], [3 * SB, 3], [W, RB]],
                    extra_offset=2 * SB + W - 1), 0.0)

            prev_last = None   # pin gl order: the static scheduler
            # otherwise reorders chains by its (wrong) DMA timing model,
            # head-of-line blocking the DVE queue on late inputs.

            # gl0/gl1 use mult+add chains (2 ops per arriving w plane --
            # matches the DMA delivery rate during the ramp); gl2..7 use
            # the wide tree reduction (lower instruction overhead).
            for gl in range(2):
                acc_t = opool.tile([128, SB], F32, tag="o", name="acc_t")
                av = _ap(acc_t[:], [["P", 128], [1, SB]])
                for k in range(9):
                    xvw = _ap(x_sb[:], [["P", 128], [W, RB], [1, W]],
                              extra_offset=2 + gl * XGL
                              + (k // 3) * W + (k % 3) - 1)
                    wvw = _ap(w_sb[:], [["P", 128], [1, SB]],
                              extra_offset=k * SB)
                    if k == 0:
                        m = nc.vector.tensor_mul(out=av, in0=xvw, in1=wvw)
                        if prev_last is not None:
                            add_dep_helper(m.ins, prev_last.ins, sync=False,
                                           reason="gl chain order")
                    else:
                        tmp = qpool.tile([128, SB], F32, tag="tmp",
                                         name="tmp")
                        tv = _ap(tmp[:], [["P", 128], [1, SB]])
                        nc.vector.tensor_mul(out=tv, in0=xvw, in1=wvw)
                        prev_last = nc.vector.tensor_add(out=av, in0=av,
                                                         in1=tv)
                out_dma(gl, _ap(acc_t[:], [["P", 128], [1, SB]]))

            for gl in range(2, GL):
                q = qpool.tile([128, 9 * SB], F32, tag="q", name="q")
                for k in range(9):
                    xvw = _ap(x_sb[:], [["P", 128], [W, RB], [1, W]],
                              extra_offset=2 + gl * XGL
                              + (k // 3) * W + (k % 3) - 1)
                    wvw = _ap(w_sb[:], [["P", 128], [1, SB]],
                              extra_offset=k * SB)
                    qv = _ap(q[:], [["P", 128], [1, SB]],
                             extra_offset=k * SB)
                    m = nc.vector.tensor_mul(out=qv, in0=xvw, in1=wvw)
                    if k == 0 and prev_last is not None:
                        add_dep_helper(m.ins, prev_last.ins, sync=False,
                                       reason="gl chain order")
                # binary tree, in place: [1..5)+=[5..9); [1..3)+=[3..5);
                # [1..2)+=[2..3); out = [0..1)+[1..2)
                nc.vector.tensor_add(
                    out=_ap(q[:], [["P", 128], [1, 4 * SB]],
                            extra_offset=SB),
                    in0=_ap(q[:], [["P", 128], [1, 4 * SB]],
                            extra_offset=SB),
                    in1=_ap(q[:], [["P", 128], [1, 4 * SB]],
                            extra_offset=5 * SB))
                nc.vector.tensor_add(
                    out=_ap(q[:], [["P", 128], [1, 2 * SB]],
                            extra_offset=SB),
                    in0=_ap(q[:], [["P", 128], [1, 2 * SB]],
                            extra_offset=SB),
                    in1=_ap(q[:], [["P", 128], [1, 2 * SB]],
                            extra_offset=3 * SB))
                nc.vector.tensor_add(
                    out=_ap(q[:], [["P", 128], [1, SB]], extra_offset=SB),
                    in0=_ap(q[:], [["P", 128], [1, SB]], extra_offset=SB),
                    in1=_ap(q[:], [["P", 128], [1, SB]],
                            extra_offset=2 * SB))
                o_t = opool.tile([128, SB], F32, tag="o", name="o_t")
                prev_last = nc.vector.tensor_add(
                    out=_ap(o_t[:], [["P", 128], [1, SB]]),
                    in0=_ap(q[:], [["P", 128], [1, SB]]),
                    in1=_ap(q[:], [["P", 128], [1, SB]], extra_offset=SB))
                out_dma(gl, _ap(o_t[:], [["P", 128], [1, SB]]))

    nc.compile()
    return nc


_NC_CACHE = None


def _get_nc():
    global _NC_CACHE
    if _NC_CACHE is None:
        _NC_CACHE = build_program()
    return _NC_CACHE


def kernel(input, weight):
    """input: [8,64,128,128] f32, weight: [8,8,9,16384] f32 ->
    [8,64,128,128] f32."""
    from concourse.bass_utils import run_bass_kernel_spmd

    x = np.ascontiguousarray(np.asarray(input, dtype=np.float32))
    w = np.ascontiguousarray(np.asarray(weight, dtype=np.float32))
    N = x.shape[0]
    nc = _get_nc()
    in_maps = [{"x": x[i].reshape(C, S), "w": w[i].reshape(CW, 9, S)}
               for i in range(N)]
    res = run_bass_kernel_spmd(nc, in_maps, core_ids=list(range(N)))
    out = np.stack([res.results[i]["out"].reshape(C, H, W) for i in range(N)])
    return out
